# revision 1
# baseline (speedup 1.0000x reference)
"""Trainium2 Bass kernel for nn_MHSA_37821482008969 (2D rel-pos MHSA).

Strategy: data-parallel over batch (16 batches -> 8 cores x 2). Per (batch,
head) unit, attention is computed fully transposed: S^T = K^T@Q tiles with
y (keys) on partitions, so softmax-normalization sums come from a ones-vector
matmul on PE, the attn matmul needs no transposes of exp(S), and the output
lands directly in the channel-major layout the conv output wants.

Rel-pos biases are folded into the logits accumulation as one extra K=64
matmul per tile: lhsT is a constant 0/1 selector, rhs is the skewed rel-logit
table built via a DRAM round-trip (regular strided APs implement the
rel->abs skew) plus two PE transposes for the width term.

All matmul operands are bf16 (fp32 PSUM accumulation); softmax skips the
row-max subtraction (logits are ~N(0,1), |logit| < 7, exp is safe in fp32).
"""
import numpy as np
import ml_dtypes

import concourse.bass as bass
import concourse.mybir as mybir
import concourse.tile as tile
import concourse.bacc as bacc
from concourse.bass_utils import run_bass_kernel_spmd

bf16 = ml_dtypes.bfloat16
FP32 = mybir.dt.float32
BF16 = mybir.dt.bfloat16

HEADS, D, F, DIM = 4, 128, 32, 512
L = F * F           # 1024
B_PER_CORE = 2
N_CORES = 8
AF = mybir.ActivationFunctionType

_cache = {}


def _build():
    nc = bacc.Bacc("TRN2", target_bir_lowering=False, debug=False,
                   num_devices=N_CORES)
    xin = nc.dram_tensor("xin", [B_PER_CORE, 4, 128, L], BF16, kind="ExternalInput").ap()
    wqt = nc.dram_tensor("wqt", [4, 128, DIM], BF16, kind="ExternalInput").ap()
    wkt = nc.dram_tensor("wkt", [4, 128, DIM], BF16, kind="ExternalInput").ap()
    wvt = nc.dram_tensor("wvt", [4, 128, DIM], BF16, kind="ExternalInput").ap()
    relwt = nc.dram_tensor("relwt", [128, 63], BF16, kind="ExternalInput").ap()
    relht = nc.dram_tensor("relht", [128, 63], BF16, kind="ExternalInput").ap()
    sel = nc.dram_tensor("sel", [64, 8 * 128], BF16, kind="ExternalInput").ap()
    ones_col = nc.dram_tensor("ones_col", [128, 1], BF16, kind="ExternalInput").ap()
    ones_row = nc.dram_tensor("ones_row", [1, 128], BF16, kind="ExternalInput").ap()
    ident = nc.dram_tensor("ident", [128, 128], BF16, kind="ExternalInput").ap()
    out = nc.dram_tensor("out", [B_PER_CORE, DIM, L], FP32, kind="ExternalOutput").ap()

    from contextlib import ExitStack
    ctx = ExitStack()
    with tile.TileContext(nc) as tc, ctx:
        consts = ctx.enter_context(tc.tile_pool(name="consts", bufs=1))
        xpool = ctx.enter_context(tc.tile_pool(name="xpool", bufs=2))
        vtpool = ctx.enter_context(tc.tile_pool(name="vtpool", bufs=2))
        qkpool = ctx.enter_context(tc.tile_pool(name="qkpool", bufs=2))
        rwpool = ctx.enter_context(tc.tile_pool(name="rwpool", bufs=4))
        biaspool = ctx.enter_context(tc.tile_pool(name="biaspool", bufs=2))
        ptpool = ctx.enter_context(tc.tile_pool(name="ptpool", bufs=2))
        outpool = ctx.enter_context(tc.tile_pool(name="outpool", bufs=3))
        mmps = ctx.enter_context(tc.tile_pool(name="mmps", bufs=3, space="PSUM"))
        attnps = ctx.enter_context(tc.tile_pool(name="attnps", bufs=2, space="PSUM"))
        sumsps = ctx.enter_context(tc.tile_pool(name="sumsps", bufs=1, space="PSUM"))
        tpsps = ctx.enter_context(tc.tile_pool(name="tpsps", bufs=2, space="PSUM"))
        dramw = ctx.enter_context(tc.tile_pool(name="dramw", bufs=2, space="DRAM"))
        dramh = ctx.enter_context(tc.tile_pool(name="dramh", bufs=2, space="DRAM"))

        # ---- load constants ----
        def cload(ap, shape, tag):
            t = consts.tile(shape, ap.dtype, tag=tag)
            nc.sync.dma_start(t[:], ap)
            return t
        wq_sb = [cload(wqt[c], [128, DIM], f"wq{c}") for c in range(4)]
        wk_sb = [cload(wkt[c], [128, DIM], f"wk{c}") for c in range(4)]
        wv_sb = [cload(wvt[c], [128, DIM], f"wv{c}") for c in range(4)]
        relw_sb = cload(relwt, [128, 63], "relw")
        relh_sb = cload(relht, [128, 63], "relh")
        sel_sb = cload(sel, [64, 8 * 128], "sel")
        ones_c = cload(ones_col, [128, 1], "onesc")
        ones_r = cload(ones_row, [1, 128], "onesr")
        id_sb = cload(ident, [128, 128], "ident")

        for b in range(B_PER_CORE):
            x_sb = []
            for c in range(4):
                xt = xpool.tile([128, L], BF16, tag=f"x{c}")
                nc.sync.dma_start(xt[:], xin[b, c])
                x_sb.append(xt)
            # V^T for all heads: vt_sb[yt][y(128), d(512 all heads)]
            vt_sb = []
            for yt in range(8):
                ps = mmps.tile([128, DIM], FP32, tag="mm")
                for c in range(4):
                    nc.tensor.matmul(ps[:], x_sb[c][:, yt * 128:(yt + 1) * 128],
                                     wv_sb[c][:], start=(c == 0), stop=(c == 3))
                vt = vtpool.tile([128, DIM], BF16, tag=f"vt{yt}")
                nc.vector.tensor_copy(vt[:], ps[:])
                vt_sb.append(vt)

            qs, ks, biases = [], [], []
            for h in range(HEADS):
                # ---- Q, K projections: [d(128), L] layouts ----
                q_sb = qkpool.tile([128, L], BF16, tag=f"q{h}")
                k_sb = qkpool.tile([128, L], BF16, tag=f"k{h}")
                qs.append(q_sb); ks.append(k_sb)
                for dst, w in ((q_sb, wq_sb), (k_sb, wk_sb)):
                    ps0 = mmps.tile([128, 512], FP32, tag="mm")
                    ps1 = mmps.tile([128, 512], FP32, tag="mm")
                    pss = [ps0, ps1]
                    for c in range(4):
                        for n in range(2):
                            nc.tensor.matmul(pss[n][:], w[c][:, h * 128:(h + 1) * 128],
                                             x_sb[c][:, n * 512:(n + 1) * 512],
                                             start=(c == 0), stop=(c == 3))
                    for n in range(2):
                        nc.vector.tensor_copy(dst[:, n * 512:(n + 1) * 512], pss[n][:])

            for h in range(HEADS):
                q_sb = qs[h]
                # ---- rel width logits RW[q,m], bounce via DRAM, skew ----
                skw = dramw.tile([L, 64], BF16, tag="skw")
                for j in range(8):
                    ps = mmps.tile([128, 64], FP32, tag="mm")
                    nc.tensor.matmul(ps[:, 0:63], q_sb[:, j * 128:(j + 1) * 128],
                                     relw_sb[:], start=True, stop=True)
                    rw = rwpool.tile([128, 64], BF16, tag="rw")
                    nc.vector.tensor_copy(rw[:, 0:63], ps[:, 0:63])
                    nc.sync.dma_start(skw[j * 128:(j + 1) * 128, 0:63], rw[:, 0:63])
                # ---- rel height logits RH_T[m,q] -> DRAM ----
                skh = dramh.tile([64, L], BF16, tag="skh")
                rh = rwpool.tile([64, L], BF16, tag="rh")
                for n in range(2):
                    ps = mmps.tile([64, 512], FP32, tag="mm")
                    nc.tensor.matmul(ps[0:63, :], relh_sb[:],
                                     q_sb[:, n * 512:(n + 1) * 512],
                                     start=True, stop=True)
                    nc.vector.tensor_copy(rh[0:63, n * 512:(n + 1) * 512], ps[0:63, :])
                nc.sync.dma_start(skh[0:63, :], rh[0:63, :])

                # ---- skewed reads -> bias_rhs [64, L] ----
                bias_rhs = biaspool.tile([64, L], BF16, tag=f"bias{h}")
                biases.append(bias_rhs)
                wst2 = biaspool.tile([128, 256], BF16, tag="wst2")
                src_flat = skw[:].flatten()
                dst_flat = wst2[:]
                for xh in range(4):
                    srcap = bass.AP(src_flat.tensor, src_flat.offset + 31 + xh * 2048,
                                    [[63, 32], [8192, 8], [1, 32]])
                    dstap = bass.AP(dst_flat.tensor, dst_flat.offset + xh * 32 * 256,
                                    [[256, 32], [32, 8], [1, 32]])
                    nc.sync.dma_start(dstap, srcap)
                for half in range(2):
                    tps = tpsps.tile([128, 128], BF16, tag="tps")
                    nc.tensor.transpose(tps[:], wst2[:, half * 128:(half + 1) * 128],
                                        id_sb[:])
                    wst3 = biaspool.tile([128, 128], BF16, tag=f"wst3{half}")
                    nc.vector.tensor_copy(wst3[:], tps[:])
                    for jj in range(4):
                        j = half * 4 + jj
                        nc.sync.dma_start(bias_rhs[0:32, j * 128:(j + 1) * 128],
                                          wst3[jj * 32:(jj + 1) * 32, :])
                hsrc_flat = skh[:].flatten()
                hsrc = bass.AP(hsrc_flat.tensor, hsrc_flat.offset,
                               [[1024, 32], [1056, 32], [1, 32]])
                hdst_flat = bias_rhs[:]
                hdst = bass.AP(hdst_flat.tensor, hdst_flat.offset + 32 * 1024,
                               [[1024, 32], [32, 32], [1, 32]])
                nc.sync.dma_start(hdst, hsrc)

            for h in range(HEADS):
                q_sb, k_sb, bias_rhs = qs[h], ks[h], biases[h]
                # ---- attention, per 512-wide q block ----
                for n in range(2):
                    pt_sb = []
                    for yt in range(8):
                        st = mmps.tile([128, 512], FP32, tag="mm")
                        nc.tensor.matmul(st[:], k_sb[:, yt * 128:(yt + 1) * 128],
                                         q_sb[:, n * 512:(n + 1) * 512],
                                         start=True, stop=False)
                        nc.tensor.matmul(st[:], sel_sb[:, yt * 128:(yt + 1) * 128],
                                         bias_rhs[:, n * 512:(n + 1) * 512],
                                         start=False, stop=True)
                        pt = ptpool.tile([128, 512], BF16, tag=f"pt{yt}")
                        nc.scalar.activation(pt[:], st[:], AF.Exp)
                        pt_sb.append(pt)
                    sums = sumsps.tile([1, 512], FP32, tag="sums")
                    attn = attnps.tile([128, 512], FP32, tag="attn")
                    for yt in range(8):
                        nc.tensor.matmul(sums[:], ones_c[:], pt_sb[yt][:],
                                         start=(yt == 0), stop=(yt == 7))
                    for yt in range(8):
                        nc.tensor.matmul(attn[:], vt_sb[yt][:, h * 128:(h + 1) * 128],
                                         pt_sb[yt][:], start=(yt == 0), stop=(yt == 7))
                    recip = outpool.tile([1, 512], BF16, tag="recip")
                    with nc.allow_low_precision(reason="bf16 softmax recip"):
                        nc.vector.reciprocal(recip[:], sums[:])
                    bc = mmps.tile([128, 512], FP32, tag="mm")
                    nc.tensor.matmul(bc[:], ones_r[:], recip[:], start=True, stop=True)
                    bc_sb = outpool.tile([128, 512], FP32, tag="bcsb")
                    nc.scalar.activation(bc_sb[:], bc[:], AF.Identity)
                    o_sb = outpool.tile([128, 512], FP32, tag="osb")
                    nc.vector.tensor_mul(o_sb[:], attn[:], bc_sb[:])
                    nc.sync.dma_start(
                        out[b, h * 128:(h + 1) * 128, n * 512:(n + 1) * 512], o_sb[:])

    nc.compile()
    return nc


def _prep_inputs(featuremap, w_qk, w_v, rel_height, rel_width):
    scale = D ** -0.5
    wqt = np.ascontiguousarray(w_qk[:DIM].T * scale).astype(bf16).reshape(4, 128, DIM)
    wkt = np.ascontiguousarray(w_qk[DIM:].T).astype(bf16).reshape(4, 128, DIM)
    wvt = np.ascontiguousarray(w_v.T).astype(bf16).reshape(4, 128, DIM)
    relwt = np.ascontiguousarray(rel_width.T).astype(bf16)
    relht = np.ascontiguousarray(rel_height.T[:, ::-1]).astype(bf16)
    yy = np.arange(128)
    sel = np.zeros((64, 8 * 128), np.float32)
    for yt in range(8):
        sel[yy % 32, yt * 128 + yy] = 1.0
        sel[32 + 31 - (yt * 4 + yy // 32), yt * 128 + yy] = 1.0
    sel = sel.astype(bf16)
    ones_col = np.ones((128, 1), bf16)
    ones_row = np.ones((1, 128), bf16)
    ident = np.eye(128, dtype=bf16)
    common = dict(wqt=wqt, wkt=wkt, wvt=wvt, relwt=relwt, relht=relht,
                  sel=sel, ones_col=ones_col, ones_row=ones_row, ident=ident)
    xin = featuremap.reshape(16, DIM, L).astype(bf16).reshape(
        N_CORES, B_PER_CORE, 4, 128, L)
    return [dict(common, xin=np.ascontiguousarray(xin[i])) for i in range(N_CORES)]


def kernel(featuremap, w_qk, w_v, rel_height, rel_width, _trace=False, _tmpdir=None):
    if "nc" not in _cache:
        _cache["nc"] = _build()
    nc = _cache["nc"]
    in_maps = _prep_inputs(featuremap, w_qk, w_v, rel_height, rel_width)
    res = run_bass_kernel_spmd(nc, in_maps, list(range(N_CORES)),
                               trace=_trace, tmpdir=_tmpdir)
    _cache["last_result"] = res
    full = np.concatenate([res.results[i]["out"] for i in range(N_CORES)], axis=0)
    return full.reshape(16, DIM, F, F)



# revision 4
# speedup vs baseline: 1.6385x; 1.6385x over previous
"""Trainium2 Bass kernel for nn_MHSA_37821482008969 (2D rel-pos MHSA).

Strategy: data-parallel over batch (16 batches -> 8 cores x 2). Per (batch,
head) unit, attention is computed fully transposed: S^T = K^T@Q tiles with
y (keys) on partitions, so softmax-normalization sums come from a ones-vector
matmul on PE, the attn matmul needs no transposes of exp(S), and the output
lands directly in the channel-major layout the conv output wants.

Rel-pos biases are folded into the logits accumulation as one extra K=64
matmul per tile: lhsT is a constant 0/1 selector, rhs is the skewed rel-logit
table built via a DRAM round-trip (regular strided APs implement the
rel->abs skew) plus two PE transposes for the width term.

v2 restructure (perf): both batches' projections emitted up front, then the
16 attention (head, x-block) blocks run as a software pipeline with the
normalization tail lagged two blocks so the PE never stalls on DVE/ACT:
  iter j: logits+exp(j) | attn+sums(j-1) | recip(j-1) | bc+mul+store(j-2)
Softmax sums use a DVE bf16 add-tree over the exp tiles (replaces 8 PE
matmuls per block with 1), exp runs on 2-PSUM-bank [128,1024] tiles, the
reciprocal uses the fast custom-DVE approximation, and the output is stored
bf16 (host upcasts). All matmul operands are bf16 (fp32 PSUM accumulation);
softmax skips the row-max subtraction (logits are ~N(0,1), |logit| < 7).
"""
import numpy as np
import ml_dtypes

import concourse.bass as bass
import concourse.mybir as mybir
import concourse.tile as tile
import concourse.bacc as bacc
from concourse.bass_utils import run_bass_kernel_spmd

bf16 = ml_dtypes.bfloat16
FP32 = mybir.dt.float32
BF16 = mybir.dt.bfloat16

HEADS, D, F, DIM = 4, 128, 32, 512
L = F * F           # 1024
B_PER_CORE = 2
N_CORES = 8
AF = mybir.ActivationFunctionType

_cache = {}


def _build():
    nc = bacc.Bacc("TRN2", target_bir_lowering=False, debug=False,
                   num_devices=N_CORES)
    xin = nc.dram_tensor("xin", [B_PER_CORE, 4, 128, L], BF16, kind="ExternalInput").ap()
    wqt = nc.dram_tensor("wqt", [4, 128, DIM], BF16, kind="ExternalInput").ap()
    wkt = nc.dram_tensor("wkt", [4, 128, DIM], BF16, kind="ExternalInput").ap()
    wvt = nc.dram_tensor("wvt", [4, 128, DIM], BF16, kind="ExternalInput").ap()
    relwt = nc.dram_tensor("relwt", [128, 63], BF16, kind="ExternalInput").ap()
    relht = nc.dram_tensor("relht", [128, 63], BF16, kind="ExternalInput").ap()
    sel = nc.dram_tensor("sel", [64, 8 * 128], BF16, kind="ExternalInput").ap()
    ones_col = nc.dram_tensor("ones_col", [128, 1], BF16, kind="ExternalInput").ap()
    ones_row = nc.dram_tensor("ones_row", [1, 128], BF16, kind="ExternalInput").ap()
    ident = nc.dram_tensor("ident", [128, 128], BF16, kind="ExternalInput").ap()
    out = nc.dram_tensor("out", [B_PER_CORE, DIM, L], BF16, kind="ExternalOutput").ap()

    from contextlib import ExitStack
    ctx = ExitStack()
    with tile.TileContext(nc) as tc, ctx:
        consts = ctx.enter_context(tc.tile_pool(name="consts", bufs=1))
        xpool = ctx.enter_context(tc.tile_pool(name="xpool", bufs=2))
        qkpool = ctx.enter_context(tc.tile_pool(name="qkpool", bufs=2))
        vtpool = ctx.enter_context(tc.tile_pool(name="vtpool", bufs=2))
        relpool = ctx.enter_context(tc.tile_pool(name="relpool", bufs=4))
        biaspool = ctx.enter_context(tc.tile_pool(name="biaspool", bufs=2))
        ptpool = ctx.enter_context(tc.tile_pool(name="ptpool", bufs=2))
        accpool = ctx.enter_context(tc.tile_pool(name="accpool", bufs=2))
        outpool = ctx.enter_context(tc.tile_pool(name="outpool", bufs=3))
        psA = ctx.enter_context(tc.tile_pool(name="psA", bufs=2, space="PSUM"))
        psB = ctx.enter_context(tc.tile_pool(name="psB", bufs=2, space="PSUM"))
        psS = ctx.enter_context(tc.tile_pool(name="psS", bufs=1, space="PSUM"))
        psC = ctx.enter_context(tc.tile_pool(name="psC", bufs=1, space="PSUM"))
        dramw = ctx.enter_context(tc.tile_pool(name="dramw", bufs=4, space="DRAM"))
        dramh = ctx.enter_context(tc.tile_pool(name="dramh", bufs=4, space="DRAM"))

        # ---- load constants ----
        def cload(ap, shape, tag):
            t = consts.tile(shape, ap.dtype, tag=tag, name=tag)
            nc.sync.dma_start(t[:], ap)
            return t
        wq_sb = [cload(wqt[c], [128, DIM], f"wq{c}") for c in range(4)]
        wk_sb = [cload(wkt[c], [128, DIM], f"wk{c}") for c in range(4)]
        wv_sb = [cload(wvt[c], [128, DIM], f"wv{c}") for c in range(4)]
        relw_sb = cload(relwt, [128, 63], "relw")
        relh_sb = cload(relht, [128, 63], "relh")
        sel_sb = cload(sel, [64, 8 * 128], "sel")
        ones_c = cload(ones_col, [128, 1], "onesc")
        ones_r = cload(ones_row, [1, 128], "onesr")
        id_sb = cload(ident, [128, 128], "ident")

        # ---- per-batch state ----
        x_all = [[None] * 4 for _ in range(B_PER_CORE)]
        q_all = [[None] * HEADS for _ in range(B_PER_CORE)]
        k_all = [[None] * HEADS for _ in range(B_PER_CORE)]
        vt_all = [[None] * 8 for _ in range(B_PER_CORE)]
        bias_all = [[None] * HEADS for _ in range(B_PER_CORE)]

        # prefetch both batches' featuremaps immediately
        for b in range(B_PER_CORE):
            for c in range(4):
                xt = xpool.tile([128, L], BF16, tag=f"x{c}", name=f"x{b}_{c}")
                nc.sync.dma_start(xt[:], xin[b, c])
                x_all[b][c] = xt

        # W transposes need the wst2 tiles; keep per-head refs
        wst2_all = [[None] * HEADS for _ in range(B_PER_CORE)]

        def emit_proj2(b):
            x_sb = x_all[b]
            for h in range(HEADS):
                for dst_list, w in ((q_all, wq_sb), (k_all, wk_sb)):
                    ps = psA.tile([128, 2 * DIM], FP32, tag="st", name=f"qk{b}_{h}")
                    for c in range(4):
                        lhsT = w[c][:, h * 128:(h + 1) * 128]
                        nc.tensor.matmul(ps[:, 0:512], lhsT, x_sb[c][:, 0:512],
                                         start=(c == 0), stop=(c == 3))
                        nc.tensor.matmul(ps[:, 512:1024], lhsT, x_sb[c][:, 512:1024],
                                         start=(c == 0), stop=(c == 3))
                    dst = qkpool.tile([128, L], BF16,
                                      tag=("q" if dst_list is q_all else "k") + str(h),
                                      name=f"qk{b}_{h}")
                    nc.vector.tensor_copy(dst[:], ps[:])
                    dst_list[b][h] = dst
                q_sb = q_all[b][h]
                skw = dramw.tile([L, 64], BF16, tag="skw", name=f"skw{b}_{h}")
                for j in range(8):
                    ps = psB.tile([128, 512], FP32, tag="attn", name=f"rw{b}_{h}")
                    nc.tensor.matmul(ps[:, 0:63], q_sb[:, j * 128:(j + 1) * 128],
                                     relw_sb[:], start=True, stop=True)
                    rw = relpool.tile([128, 64], BF16, tag="rw", name=f"rw{b}_{h}")
                    nc.vector.tensor_copy(rw[:, 0:63], ps[:, 0:63])
                    nc.sync.dma_start(skw[j * 128:(j + 1) * 128, 0:63], rw[:, 0:63])
                ps2 = psA.tile([128, 2 * DIM], FP32, tag="st", name=f"rh{b}_{h}")
                nc.tensor.matmul(ps2[0:63, 0:512], relh_sb[:], q_sb[:, 0:512],
                                 start=True, stop=True)
                nc.tensor.matmul(ps2[0:63, 512:1024], relh_sb[:], q_sb[:, 512:1024],
                                 start=True, stop=True)
                rh = relpool.tile([64, L], BF16, tag="rh", name=f"rh{b}_{h}")
                nc.vector.tensor_copy(rh[0:63, :], ps2[0:63, :])
                skh = dramh.tile([64, L], BF16, tag="skh", name=f"skh{b}_{h}")
                nc.sync.dma_start(skh[0:63, :], rh[0:63, :])

                bias_rhs = biaspool.tile([64, L], BF16, tag=f"bias{h}",
                                         name=f"bias{b}_{h}")
                bias_all[b][h] = bias_rhs
                wst2 = relpool.tile([128, 256], BF16, tag="wst2", name=f"wst2{b}_{h}")
                wst2_all[b][h] = wst2
                src_flat = skw[:].flatten()
                dst_flat = wst2[:]
                for xh in range(4):
                    srcap = bass.AP(src_flat.tensor, src_flat.offset + 31 + xh * 2048,
                                    [[63, 32], [8192, 8], [1, 32]])
                    dstap = bass.AP(dst_flat.tensor, dst_flat.offset + xh * 32 * 256,
                                    [[256, 32], [32, 8], [1, 32]])
                    nc.sync.dma_start(dstap, srcap)
                hsrc_flat = skh[:].flatten()
                hsrc = bass.AP(hsrc_flat.tensor, hsrc_flat.offset,
                               [[1024, 32], [1056, 32], [1, 32]])
                hdst_flat = bias_rhs[:]
                hdst = bass.AP(hdst_flat.tensor, hdst_flat.offset + 32 * 1024,
                               [[1024, 32], [32, 32], [1, 32]])
                nc.sync.dma_start(hdst, hsrc)

            for yt in range(8):
                ps = psB.tile([128, 512], FP32, tag="attn", name=f"v{b}_{yt}")
                for c in range(4):
                    nc.tensor.matmul(ps[:], x_all[b][c][:, yt * 128:(yt + 1) * 128],
                                     wv_sb[c][:], start=(c == 0), stop=(c == 3))
                vt = vtpool.tile([128, DIM], BF16, tag=f"vt{yt}", name=f"v{b}_{yt}")
                nc.scalar.activation(vt[:], ps[:], AF.Identity)
                vt_all[b][yt] = vt

            for h in range(HEADS):
                wst2 = wst2_all[b][h]
                bias_rhs = bias_all[b][h]
                for half in range(2):
                    tps = psB.tile([128, 128], BF16, tag="attn", name=f"tp{b}_{h}")
                    nc.tensor.transpose(tps[:], wst2[:, half * 128:(half + 1) * 128],
                                        id_sb[:])
                    wst3 = relpool.tile([128, 128], BF16, tag="wst3", name=f"w3{b}_{h}")
                    nc.vector.tensor_copy(wst3[:], tps[:])
                    for jj in range(4):
                        j = half * 4 + jj
                        nc.sync.dma_start(bias_rhs[0:32, j * 128:(j + 1) * 128],
                                          wst3[jj * 32:(jj + 1) * 32, :])

        for b in range(B_PER_CORE):
            emit_proj2(b)

        # ---- attention: 16 (b, h, n) blocks, software-pipelined ----
        blocks = [(b, h, n)
                  for b in range(B_PER_CORE)
                  for h in range(HEADS)
                  for n in range(2)]
        S = [None] * len(blocks)

        def emit_logits(j):
            b, h, n = blocks[j]
            q_sb, k_sb = q_all[b][h], k_all[b][h]
            bias_rhs = bias_all[b][h]
            nsl = slice(n * 512, (n + 1) * 512)
            pts = []
            for p in range(4):
                st = psA.tile([128, 1024], FP32, tag="st", name=f"st{j}_{p}")
                for half in range(2):
                    yt = p * 2 + half
                    sl = slice(half * 512, (half + 1) * 512)
                    nc.tensor.matmul(st[:, sl], k_sb[:, yt * 128:(yt + 1) * 128],
                                     q_sb[:, nsl], start=True, stop=False)
                    nc.tensor.matmul(st[:, sl], sel_sb[:, yt * 128:(yt + 1) * 128],
                                     bias_rhs[:, nsl], start=False, stop=True)
                pt = ptpool.tile([128, 1024], BF16, tag=f"pt{p}", name=f"pt{j}_{p}")
                nc.scalar.activation(pt[:], st[:], AF.Exp)
                pts.append(pt)
            # bf16 add-tree: acc[y, x] = sum over the 8 yt tiles
            t01 = accpool.tile([128, 1024], BF16, tag="t01", name=f"t01_{j}")
            nc.vector.tensor_add(t01[:], pts[0][:], pts[1][:])
            t23 = accpool.tile([128, 1024], BF16, tag="t23", name=f"t23_{j}")
            nc.vector.tensor_add(t23[:], pts[2][:], pts[3][:])
            tsum = accpool.tile([128, 1024], BF16, tag="tsum", name=f"ts_{j}")
            nc.vector.tensor_add(tsum[:], t01[:], t23[:])
            acc = accpool.tile([128, 512], BF16, tag="acc", name=f"acc_{j}")
            nc.vector.tensor_add(acc[:], tsum[:, 0:512], tsum[:, 512:1024])
            S[j] = {"pts": pts, "acc": acc}

        def emit_attn(j):
            b, h, n = blocks[j]
            s = S[j]
            attn = psB.tile([128, 512], FP32, tag="attn", name=f"at{j}")
            for yt in range(8):
                nc.tensor.matmul(attn[:],
                                 vt_all[b][yt][:, h * 128:(h + 1) * 128],
                                 s["pts"][yt // 2][:, (yt % 2) * 512:(yt % 2 + 1) * 512],
                                 start=(yt == 0), stop=(yt == 7))
            sums = psS.tile([1, 512], FP32, tag="sums", name=f"sm{j}")
            nc.tensor.matmul(sums[:], ones_c[:], s["acc"][:], start=True, stop=True)
            recipf = accpool.tile([1, 512], FP32, tag="recipf", name=f"rf{j}")
            nc.vector.reciprocal_approx_fast(recipf[:], sums[:])
            recipb = accpool.tile([1, 512], BF16, tag="recipb", name=f"rb{j}")
            nc.vector.tensor_copy(recipb[:], recipf[:])
            s["attn"] = attn
            s["recipb"] = recipb

        def emit_norm(j):
            b, h, n = blocks[j]
            s = S[j]
            bc = psC.tile([128, 512], FP32, tag="bc", name=f"bc{j}")
            nc.tensor.matmul(bc[:], ones_r[:], s["recipb"][:], start=True, stop=True)
            bcs = outpool.tile([128, 512], BF16, tag="bcs", name=f"bcs{j}")
            nc.scalar.activation(bcs[:], bc[:], AF.Identity)
            o = outpool.tile([128, 512], BF16, tag="o", name=f"o{j}")
            with nc.allow_low_precision(reason="bf16 output"):
                nc.vector.tensor_mul(o[:], s["attn"][:], bcs[:])
            nc.sync.dma_start(out[b, h * 128:(h + 1) * 128, n * 512:(n + 1) * 512],
                              o[:])
            S[j] = None

        for j in range(len(blocks)):
            emit_logits(j)
            if j >= 1:
                emit_attn(j - 1)
            if j >= 2:
                emit_norm(j - 2)
        emit_attn(len(blocks) - 1)
        emit_norm(len(blocks) - 2)
        emit_norm(len(blocks) - 1)

    nc.compile()
    return nc


def _prep_inputs(featuremap, w_qk, w_v, rel_height, rel_width):
    scale = D ** -0.5
    wqt = np.ascontiguousarray(w_qk[:DIM].T * scale).astype(bf16).reshape(4, 128, DIM)
    wkt = np.ascontiguousarray(w_qk[DIM:].T).astype(bf16).reshape(4, 128, DIM)
    wvt = np.ascontiguousarray(w_v.T).astype(bf16).reshape(4, 128, DIM)
    relwt = np.ascontiguousarray(rel_width.T).astype(bf16)
    relht = np.ascontiguousarray(rel_height.T[:, ::-1]).astype(bf16)
    yy = np.arange(128)
    sel = np.zeros((64, 8 * 128), np.float32)
    for yt in range(8):
        sel[yy % 32, yt * 128 + yy] = 1.0
        sel[32 + 31 - (yt * 4 + yy // 32), yt * 128 + yy] = 1.0
    sel = sel.astype(bf16)
    ones_col = np.ones((128, 1), bf16)
    ones_row = np.ones((1, 128), bf16)
    ident = np.eye(128, dtype=bf16)
    common = dict(wqt=wqt, wkt=wkt, wvt=wvt, relwt=relwt, relht=relht,
                  sel=sel, ones_col=ones_col, ones_row=ones_row, ident=ident)
    xin = featuremap.reshape(16, DIM, L).astype(bf16).reshape(
        N_CORES, B_PER_CORE, 4, 128, L)
    return [dict(common, xin=np.ascontiguousarray(xin[i])) for i in range(N_CORES)]


def kernel(featuremap, w_qk, w_v, rel_height, rel_width, _trace=False, _tmpdir=None):
    if "nc" not in _cache:
        _cache["nc"] = _build()
    nc = _cache["nc"]
    in_maps = _prep_inputs(featuremap, w_qk, w_v, rel_height, rel_width)
    res = run_bass_kernel_spmd(nc, in_maps, list(range(N_CORES)),
                               trace=_trace, tmpdir=_tmpdir)
    _cache["last_result"] = res
    full = np.concatenate([res.results[i]["out"] for i in range(N_CORES)], axis=0)
    return full.astype(np.float32).reshape(16, DIM, F, F)


# revision 11
# speedup vs baseline: 1.9122x; 1.1670x over previous
"""Trainium2 Bass kernel for nn_MHSA_37821482008969 (2D rel-pos MHSA).

Strategy: data-parallel over batch (16 batches -> 8 cores x 2). Per (batch,
head) unit, attention is computed fully transposed: S^T = K^T@Q tiles with
y (keys) on partitions, so softmax-normalization sums come from a ones-vector
matmul on PE, the attn matmul needs no transposes of exp(S), and the output
lands directly in the channel-major layout the conv output wants.

Rel-pos biases are folded into the logits accumulation as one extra K=64
matmul per tile: lhsT is a constant 0/1 selector, rhs is the skewed rel-logit
table built via a DRAM round-trip (regular strided APs implement the
rel->abs skew) plus two PE transposes for the width term.

v3 perf structure:
- both batches' projections emitted up front; rel-table building lags one
  head behind the Q/K projections so PE never waits on the PSUM->SBUF casts.
- 16 attention (head, x-block) blocks run as a software pipeline with the
  normalization tail lagged two blocks:
    iter j: logits+exp(j) | attn+sums+recip(j-1) | bcast+mul+store(j-2)
- softmax sums via DVE bf16 add-tree (1 PE matmul instead of 8 per block);
  reciprocal via the fast custom-DVE approximation; the 1/sum row is
  broadcast across partitions by a stride-0 DRAM-read DMA (no PE matmul).
- rel-width logits for all 8 x-tiles go into one PSUM bank, one cast, one
  3D-strided DMA; DMA dispatch is spread across the idle GpSimd queue.
- exp runs on 2-PSUM-bank [128,1024] tiles; output stored bf16, host upcasts.
All matmul operands are bf16 (fp32 PSUM accumulation); softmax skips the
row-max subtraction (logits are ~N(0,1), |logit| < 7, exp is safe in fp32).
"""
import numpy as np
import ml_dtypes

import concourse.bass as bass
import concourse.mybir as mybir
import concourse.tile as tile
import concourse.bacc as bacc
from concourse.bass_utils import run_bass_kernel_spmd

bf16 = ml_dtypes.bfloat16
FP32 = mybir.dt.float32
BF16 = mybir.dt.bfloat16

HEADS, D, F, DIM = 4, 128, 32, 512
L = F * F           # 1024
B_PER_CORE = 2
N_CORES = 8
AF = mybir.ActivationFunctionType

_cache = {}


def _build():
    nc = bacc.Bacc("TRN2", target_bir_lowering=False, debug=False,
                   num_devices=N_CORES)
    xin = nc.dram_tensor("xin", [B_PER_CORE, 4, 128, L], BF16, kind="ExternalInput").ap()
    wqt = nc.dram_tensor("wqt", [4, 128, DIM], BF16, kind="ExternalInput").ap()
    wkt = nc.dram_tensor("wkt", [4, 128, DIM], BF16, kind="ExternalInput").ap()
    wvt = nc.dram_tensor("wvt", [4, 128, DIM], BF16, kind="ExternalInput").ap()
    relwt = nc.dram_tensor("relwt", [128, 63], BF16, kind="ExternalInput").ap()
    relht = nc.dram_tensor("relht", [128, 63], BF16, kind="ExternalInput").ap()
    sel = nc.dram_tensor("sel", [64, 8 * 128], BF16, kind="ExternalInput").ap()
    ones_col = nc.dram_tensor("ones_col", [128, 1], BF16, kind="ExternalInput").ap()
    ident = nc.dram_tensor("ident", [128, 128], BF16, kind="ExternalInput").ap()
    out = nc.dram_tensor("out", [B_PER_CORE, DIM, L], BF16, kind="ExternalOutput").ap()

    from contextlib import ExitStack
    ctx = ExitStack()
    with tile.TileContext(nc) as tc, ctx:
        consts = ctx.enter_context(tc.tile_pool(name="consts", bufs=1))
        xpool = ctx.enter_context(tc.tile_pool(name="xpool", bufs=2))
        qkpool = ctx.enter_context(tc.tile_pool(name="qkpool", bufs=2))
        vtpool = ctx.enter_context(tc.tile_pool(name="vtpool", bufs=2))
        relpool = ctx.enter_context(tc.tile_pool(name="relpool", bufs=4))
        biaspool = ctx.enter_context(tc.tile_pool(name="biaspool", bufs=2))
        ptpool = ctx.enter_context(tc.tile_pool(name="ptpool", bufs=2))
        accpool = ctx.enter_context(tc.tile_pool(name="accpool", bufs=2))
        outpool = ctx.enter_context(tc.tile_pool(name="outpool", bufs=3))
        psA = ctx.enter_context(tc.tile_pool(name="psA", bufs=2, space="PSUM"))
        psB = ctx.enter_context(tc.tile_pool(name="psB", bufs=2, space="PSUM"))
        psS = ctx.enter_context(tc.tile_pool(name="psS", bufs=1, space="PSUM"))
        dramw = ctx.enter_context(tc.tile_pool(name="dramw", bufs=4, space="DRAM"))
        dramh = ctx.enter_context(tc.tile_pool(name="dramh", bufs=4, space="DRAM"))
        dramr = ctx.enter_context(tc.tile_pool(name="dramr", bufs=2, space="DRAM"))

        # ---- constants: spread DMA dispatch across engines so the first
        #      projection matmuls aren't stuck behind one serial queue ----
        def cload(eng, ap, shape, tag):
            t = consts.tile(shape, ap.dtype, tag=tag, name=tag)
            eng.dma_start(t[:], ap)
            return t
        wq_sb = [cload(nc.gpsimd, wqt[c], [128, DIM], f"wq{c}") for c in range(4)]
        x_all = [[None] * 4 for _ in range(B_PER_CORE)]
        for b in range(B_PER_CORE):
            for c in range(4):
                xt = xpool.tile([128, L], BF16, tag=f"x{c}", name=f"x{b}_{c}")
                nc.sync.dma_start(xt[:], xin[b, c])
                x_all[b][c] = xt
        wk_sb = [cload(nc.gpsimd, wkt[c], [128, DIM], f"wk{c}") for c in range(4)]
        wv_sb = [cload(nc.gpsimd, wvt[c], [128, DIM], f"wv{c}") for c in range(4)]
        relw_sb = cload(nc.scalar, relwt, [128, 63], "relw")
        relh_sb = cload(nc.scalar, relht, [128, 63], "relh")
        sel_sb = cload(nc.scalar, sel, [64, 8 * 128], "sel")
        ones_c = cload(nc.scalar, ones_col, [128, 1], "onesc")
        id_sb = cload(nc.scalar, ident, [128, 128], "ident")

        q_all = [[None] * HEADS for _ in range(B_PER_CORE)]
        k_all = [[None] * HEADS for _ in range(B_PER_CORE)]
        vt_all = [[None] * 4 for _ in range(B_PER_CORE)]
        bias_all = [[None] * HEADS for _ in range(B_PER_CORE)]
        wst2_all = [[None] * HEADS for _ in range(B_PER_CORE)]

        def emit_qk(b, h):
            x_sb = x_all[b]
            for dst_list, w in ((q_all, wq_sb), (k_all, wk_sb)):
                ps = psA.tile([128, 2 * DIM], FP32, tag="st", name=f"qk{b}_{h}")
                for c in range(4):
                    lhsT = w[c][:, h * 128:(h + 1) * 128]
                    nc.tensor.matmul(ps[:, 0:512], lhsT, x_sb[c][:, 0:512],
                                     start=(c == 0), stop=(c == 3))
                    nc.tensor.matmul(ps[:, 512:1024], lhsT, x_sb[c][:, 512:1024],
                                     start=(c == 0), stop=(c == 3))
                dst = qkpool.tile([128, L], BF16,
                                  tag=("q" if dst_list is q_all else "k") + str(h),
                                  name=f"qk{b}_{h}")
                nc.vector.tensor_copy(dst[:], ps[:])
                dst_list[b][h] = dst

        def emit_rel(b, h):
            q_sb = q_all[b][h]
            # ---- rel width logits RW[x, m]: 8 x-tiles into one PSUM bank ----
            psrw = psB.tile([128, 512], FP32, tag="attn", name=f"rw{b}_{h}")
            for j in range(8):
                nc.tensor.matmul(psrw[:, j * 64:j * 64 + 63],
                                 q_sb[:, j * 128:(j + 1) * 128],
                                 relw_sb[:], start=True, stop=True)
            rwall = relpool.tile([128, 512], BF16, tag="rw", name=f"rw{b}_{h}")
            nc.vector.tensor_copy(rwall[:], psrw[:])
            skw = dramw.tile([L, 64], BF16, tag="skw", name=f"skw{b}_{h}")
            rwf = rwall[:].flatten()
            swf = skw[:].flatten()
            nc.gpsimd.dma_start(
                bass.AP(swf.tensor, swf.offset, [[64, 128], [8192, 8], [1, 63]]),
                bass.AP(rwf.tensor, rwf.offset, [[512, 128], [64, 8], [1, 63]]))
            # ---- rel height logits RH_T[m, x] -> DRAM ([64, L]) ----
            ps2 = psA.tile([128, 2 * DIM], FP32, tag="st", name=f"rh{b}_{h}")
            nc.tensor.matmul(ps2[0:63, 0:512], relh_sb[:], q_sb[:, 0:512],
                             start=True, stop=True)
            nc.tensor.matmul(ps2[0:63, 512:1024], relh_sb[:], q_sb[:, 512:1024],
                             start=True, stop=True)
            rh = relpool.tile([64, L], BF16, tag="rh", name=f"rh{b}_{h}")
            nc.scalar.activation(rh[0:63, :], ps2[0:63, :], AF.Identity)
            skh = dramh.tile([64, L], BF16, tag="skh", name=f"skh{b}_{h}")
            nc.gpsimd.dma_start(skh[0:63, :], rh[0:63, :])

            # ---- skewed reads: W -> wst2 (pre-transpose), H -> bias rows 32:64
            bias_rhs = biaspool.tile([64, L], BF16, tag=f"bias{h}",
                                     name=f"bias{b}_{h}")
            bias_all[b][h] = bias_rhs
            wst2 = relpool.tile([128, 256], BF16, tag="wst2", name=f"wst2{b}_{h}")
            wst2_all[b][h] = wst2
            src_flat = skw[:].flatten()
            dst_flat = wst2[:]
            for xh in range(4):
                srcap = bass.AP(src_flat.tensor, src_flat.offset + 31 + xh * 2048,
                                [[63, 32], [8192, 8], [1, 32]])
                dstap = bass.AP(dst_flat.tensor, dst_flat.offset + xh * 32 * 256,
                                [[256, 32], [32, 8], [1, 32]])
                nc.gpsimd.dma_start(dstap, srcap)
            hsrc_flat = skh[:].flatten()
            hsrc = bass.AP(hsrc_flat.tensor, hsrc_flat.offset,
                           [[1024, 32], [1056, 32], [1, 32]])
            hdst_flat = bias_rhs[:]
            hdst = bass.AP(hdst_flat.tensor, hdst_flat.offset + 32 * 1024,
                           [[1024, 32], [32, 32], [1, 32]])
            nc.gpsimd.dma_start(hdst, hsrc)

        def emit_vt(b):
            # V^T pairs: vt2[p][y(128), (yt half)*512 + d] for all 4 heads
            for p2 in range(4):
                ps = psA.tile([128, 2 * DIM], FP32, tag="st", name=f"v{b}_{p2}")
                for half in range(2):
                    yt = p2 * 2 + half
                    sl = slice(half * 512, (half + 1) * 512)
                    for c in range(4):
                        nc.tensor.matmul(ps[:, sl],
                                         x_all[b][c][:, yt * 128:(yt + 1) * 128],
                                         wv_sb[c][:], start=(c == 0), stop=(c == 3))
                vt2 = vtpool.tile([128, 2 * DIM], BF16, tag=f"vt{p2}",
                                  name=f"v{b}_{p2}")
                nc.scalar.activation(vt2[:], ps[:], AF.Identity)
                vt_all[b][p2] = vt2

        def emit_wtrans(b):
            for h in range(HEADS):
                wst2 = wst2_all[b][h]
                bias_rhs = bias_all[b][h]
                bf = bias_rhs[:]
                for half in range(2):
                    tps = psB.tile([128, 128], BF16, tag="attn", name=f"tp{b}_{h}")
                    nc.tensor.transpose(tps[:], wst2[:, half * 128:(half + 1) * 128],
                                        id_sb[:])
                    wst3 = relpool.tile([128, 128], BF16, tag="wst3", name=f"w3{b}_{h}")
                    nc.vector.tensor_copy(wst3[:], tps[:])
                    for jj in range(4):
                        j = half * 4 + jj
                        nc.gpsimd.dma_start(bias_rhs[0:32, j * 128:(j + 1) * 128],
                                            wst3[jj * 32:(jj + 1) * 32, :])

        for b in range(B_PER_CORE):
            emit_qk(b, 0)
            for h in range(1, HEADS):
                emit_qk(b, h)
                emit_rel(b, h - 1)
            emit_rel(b, HEADS - 1)
            emit_vt(b)
            emit_wtrans(b)

        # ---- attention: 16 (b, h, n) blocks, software-pipelined ----
        blocks = [(b, h, n)
                  for b in range(B_PER_CORE)
                  for h in range(HEADS)
                  for n in range(2)]
        S = [None] * len(blocks)

        def emit_logits(j):
            b, h, n = blocks[j]
            q_sb, k_sb = q_all[b][h], k_all[b][h]
            bias_rhs = bias_all[b][h]
            nsl = slice(n * 512, (n + 1) * 512)
            pts = []
            for p in range(4):
                st = psA.tile([128, 1024], FP32, tag="st", name=f"st{j}_{p}")
                for half in range(2):
                    yt = p * 2 + half
                    sl = slice(half * 512, (half + 1) * 512)
                    nc.tensor.matmul(st[:, sl], k_sb[:, yt * 128:(yt + 1) * 128],
                                     q_sb[:, nsl], start=True, stop=False)
                    nc.tensor.matmul(st[:, sl], sel_sb[:, yt * 128:(yt + 1) * 128],
                                     bias_rhs[:, nsl], start=False, stop=True)
                pt = ptpool.tile([128, 1024], BF16, tag=f"pt{p}", name=f"pt{j}_{p}")
                nc.scalar.activation(pt[:], st[:], AF.Exp)
                pts.append(pt)
            # bf16 add-tree: acc[y, x] = sum over the 8 yt tiles
            t01 = accpool.tile([128, 1024], BF16, tag="t01", name=f"t01_{j}")
            nc.vector.tensor_add(t01[:], pts[0][:], pts[1][:])
            t23 = accpool.tile([128, 1024], BF16, tag="t23", name=f"t23_{j}")
            nc.vector.tensor_add(t23[:], pts[2][:], pts[3][:])
            tsum = accpool.tile([128, 1024], BF16, tag="tsum", name=f"ts_{j}")
            nc.vector.tensor_add(tsum[:], t01[:], t23[:])
            acc = accpool.tile([128, 512], BF16, tag="acc", name=f"acc_{j}")
            nc.vector.tensor_add(acc[:], tsum[:, 0:512], tsum[:, 512:1024])
            S[j] = {"pts": pts, "acc": acc}

        def emit_attn(j):
            b, h, n = blocks[j]
            s = S[j]
            attn = psB.tile([128, 512], FP32, tag="attn", name=f"at{j}")
            for yt in range(8):
                nc.tensor.matmul(attn[:],
                                 vt_all[b][yt // 2][:, (yt % 2) * 512 + h * 128:
                                                    (yt % 2) * 512 + (h + 1) * 128],
                                 s["pts"][yt // 2][:, (yt % 2) * 512:(yt % 2 + 1) * 512],
                                 start=(yt == 0), stop=(yt == 7))
            sums = psS.tile([1, 512], FP32, tag="sums", name=f"sm{j}")
            nc.tensor.matmul(sums[:], ones_c[:], s["acc"][:], start=True, stop=True)
            recipf = accpool.tile([1, 512], FP32, tag="recipf", name=f"rf{j}")
            nc.vector.reciprocal_approx_fast(recipf[:], sums[:])
            drc = dramr.tile([1, 512], FP32, tag="drc", name=f"drc{j}")
            nc.gpsimd.dma_start(drc[:], recipf[:])
            s["attn"] = attn
            s["drc"] = drc

        def emit_norm(j):
            b, h, n = blocks[j]
            s = S[j]
            bcs = outpool.tile([128, 512], FP32, tag="bcs", name=f"bcs{j}")
            df = s["drc"][:].flatten()
            nc.gpsimd.dma_start(bcs[:],
                                bass.AP(df.tensor, df.offset, [[0, 128], [1, 512]]))
            o = outpool.tile([128, 512], BF16, tag="o", name=f"o{j}")
            with nc.allow_low_precision(reason="bf16 output"):
                nc.vector.tensor_mul(o[:], s["attn"][:], bcs[:])
            nc.sync.dma_start(out[b, h * 128:(h + 1) * 128, n * 512:(n + 1) * 512],
                              o[:])
            S[j] = None

        for j in range(len(blocks)):
            emit_logits(j)
            if j >= 1:
                emit_attn(j - 1)
            if j >= 2:
                emit_norm(j - 2)
        emit_attn(len(blocks) - 1)
        emit_norm(len(blocks) - 2)
        emit_norm(len(blocks) - 1)

    nc.compile()
    return nc


def _prep_inputs(featuremap, w_qk, w_v, rel_height, rel_width):
    scale = D ** -0.5
    wqt = np.ascontiguousarray(w_qk[:DIM].T * scale).astype(bf16).reshape(4, 128, DIM)
    wkt = np.ascontiguousarray(w_qk[DIM:].T).astype(bf16).reshape(4, 128, DIM)
    wvt = np.ascontiguousarray(w_v.T).astype(bf16).reshape(4, 128, DIM)
    relwt = np.ascontiguousarray(rel_width.T).astype(bf16)
    relht = np.ascontiguousarray(rel_height.T[:, ::-1]).astype(bf16)
    yy = np.arange(128)
    sel = np.zeros((64, 8 * 128), np.float32)
    for yt in range(8):
        sel[yy % 32, yt * 128 + yy] = 1.0
        sel[32 + 31 - (yt * 4 + yy // 32), yt * 128 + yy] = 1.0
    sel = sel.astype(bf16)
    ones_col = np.ones((128, 1), bf16)
    ident = np.eye(128, dtype=bf16)
    common = dict(wqt=wqt, wkt=wkt, wvt=wvt, relwt=relwt, relht=relht,
                  sel=sel, ones_col=ones_col, ident=ident)
    xin = featuremap.reshape(16, DIM, L).astype(bf16).reshape(
        N_CORES, B_PER_CORE, 4, 128, L)
    return [dict(common, xin=np.ascontiguousarray(xin[i])) for i in range(N_CORES)]


def kernel(featuremap, w_qk, w_v, rel_height, rel_width, _trace=False, _tmpdir=None):
    if "nc" not in _cache:
        _cache["nc"] = _build()
    nc = _cache["nc"]
    in_maps = _prep_inputs(featuremap, w_qk, w_v, rel_height, rel_width)
    res = run_bass_kernel_spmd(nc, in_maps, list(range(N_CORES)),
                               trace=_trace, tmpdir=_tmpdir)
    _cache["last_result"] = res
    full = np.concatenate([res.results[i]["out"] for i in range(N_CORES)], axis=0)
    return full.astype(np.float32).reshape(16, DIM, F, F)
